# revision 1
# baseline (speedup 1.0000x reference)
"""Biased multi-head attention block (LayerNorm -> QKV -> attn+bias softmax -> out proj)
on 8 Trainium2 NeuronCores, data-parallel over the batch dimension (one batch element
per core).

Per-core device kernel layout strategy:
  - LayerNorm in [token, dim] layout (bn_stats/bn_aggr + tensor_scalar), then PE
    transpose to xnT [dim, token]; the V projection for each pair of token
    tiles is interleaved right behind their LayerNorm so the PE stays busy.
  - V lands in [token, feat] layout (xnT stationary, weights moving) with an
    extra all-ones column per head, so the attention row-sums (softmax
    denominators) fall out of the same matmul that computes attn @ V.
  - Q,K are projected into qT/kT [feat, token] (weights stationary, xnT
    moving) — and each head pair's projection matmuls are slotted into the
    PREVIOUS pair's attention steps, filling the PE bubbles left by the
    exp latency so the tensor engine stays dense (keeps the HAM clock gate
    at 2.4 GHz) and the projection costs no extra wall-clock.
  - Attention is computed transposed per head: simT[j, i] = k_h^T q_h on PSUM;
    exp() on the scalar engine directly out of PSUM with the 1/8 head scale
    folded into the activation's affine prestep; the additive attention bias
    becomes a vector-engine multiply by host-precomputed exp(bias)^T in bf16
    (exp(a+b) = exp(a)exp(b)) so no tensor-engine work is spent on it.
  - attn_exp^T is the moving operand of outT_h = [v_h|1]^T @ expT.
  - Normalization (divide by row sums) is applied to outT via a
    selection-matrix matmul broadcast + one elementwise multiply per k-tile.
  - Final projection y = outT^T @ w_out with outT stationary.
  - All fp32 matmul operands are typed float32r end-to-end (full-rate PE);
    attention probabilities/values/bias run in bf16.
  - One shared 4-slot PSUM pool (2 banks per slot = all 8 banks) carries every
    accumulation; in attention steady state the slots hold av, the projection
    accumulator, and two cycling sim tiles.

Measured on hardware: ~396-403us exec per core (8 cores in parallel),
rel err ~3e-3 vs the fp32 reference.
"""

import os

import numpy as np
import ml_dtypes

import concourse.bacc as bacc
import concourse.bass as bass
import concourse.mybir as mybir
import concourse.tile as tile
from concourse.bass_utils import run_bass_kernel_spmd
from concourse.masks import make_identity

B = 8
N = 1024
DIM = 1024
HEADS = 16
DH = 64
INNER = HEADS * DH
P = 128
NT = N // P          # token tiles
KT = DIM // P        # contraction tiles
PAIRS = HEADS // 2   # head pairs (one qT/kT feature tile each)
EPS = 1e-5
SCALE = DH ** -0.5   # 0.125, exact in fp32

F32 = mybir.dt.float32
F32R = mybir.dt.float32r
BF16 = mybir.dt.bfloat16
AF = mybir.ActivationFunctionType

_BUILD_CACHE = {}


def r(ap):
    """View an fp32 AP as float32r for full-rate PE matmuls."""
    return ap.bitcast(F32R)


def _maybe_enable_ldw_opt():
    """Opt-in: rewrite walrus args so LDWEIGHTS can use the background weight
    buffer (hidden behind the running matmul). bass_utils hardcodes
    --enable-ldw-opt=false; intercept its run_command to flip it."""
    if not bool(int(os.environ.get("BA_LDW_OPT", "0"))):
        return
    import concourse.bass_utils as _bu

    if getattr(_bu.run_command, "_ldw_patched", False):
        return
    _orig = _bu.run_command

    def _patched(argv, **kwargs):
        argv = [
            a.replace("--enable-ldw-opt=false", "--enable-ldw-opt=true")
            if isinstance(a, str)
            else a
            for a in argv
        ]
        return _orig(argv, **kwargs)

    _patched._ldw_patched = True
    _bu.run_command = _patched


def _build(apply_gamma: bool, apply_beta: bool):
    key = (apply_gamma, apply_beta)
    if key in _BUILD_CACHE:
        return _BUILD_CACHE[key]
    _maybe_enable_ldw_opt()

    nc = bacc.Bacc("TRN2", target_bir_lowering=False, debug=False)

    x_d = nc.dram_tensor("x", [N, DIM], F32, kind="ExternalInput")
    wqk_d = nc.dram_tensor("wqk", [PAIRS, P, KT, 2 * P], F32R, kind="ExternalInput")
    wv_d = nc.dram_tensor("wv", [KT, P, DIM], F32R, kind="ExternalInput")
    wo_d = nc.dram_tensor("wo", [P, KT, DIM], F32R, kind="ExternalInput")
    bias_d = nc.dram_tensor("biasT", [HEADS, NT, P, N], BF16, kind="ExternalInput")
    sel_d = nc.dram_tensor("sel", [HEADS, KT * P], F32R, kind="ExternalInput")
    gamma_d = beta_d = None
    if apply_gamma:
        gamma_d = nc.dram_tensor("gamma", [DIM], F32, kind="ExternalInput")
    if apply_beta:
        beta_d = nc.dram_tensor("beta", [DIM], F32, kind="ExternalInput")
    y_d = nc.dram_tensor("y", [N, DIM], F32, kind="ExternalOutput")

    with tile.TileContext(nc) as tc:
        from contextlib import ExitStack

        with ExitStack() as ctx:
            consts = ctx.enter_context(tc.tile_pool(name="consts", bufs=1))
            xpool = ctx.enter_context(tc.tile_pool(name="xpool", bufs=3))
            stats = ctx.enter_context(tc.tile_pool(name="stats", bufs=4))
            bigp = ctx.enter_context(tc.tile_pool(name="bigp", bufs=1))
            vpool = ctx.enter_context(tc.tile_pool(name="vpool", bufs=NT))
            wstream = ctx.enter_context(tc.tile_pool(name="wstream", bufs=3))
            qkpool = ctx.enter_context(tc.tile_pool(name="qkpool", bufs=4))
            epool = ctx.enter_context(tc.tile_pool(name="epool", bufs=6))
            bpool = ctx.enter_context(tc.tile_pool(name="bpool", bufs=6))
            opool = ctx.enter_context(tc.tile_pool(name="opool", bufs=KT))
            # One shared PSUM pool: 4 slots sized [128, 1024] fp32 (2 banks
            # each = all 8 banks). In the attention steady state the slots
            # hold av_h0, av_h1, sim_h0(jt), sim_h1(jt).
            pspool = ctx.enter_context(
                tc.tile_pool(name="pspool", bufs=4, space="PSUM")
            )

            ident = consts.tile([P, P], F32, name="ident")
            make_identity(nc, ident)
            eps_t = consts.tile([P, 1], F32, name="eps_t")
            nc.vector.memset(eps_t, EPS)
            # Selection matrix: S[h, kt*P + c] = 1 iff row block (kt, c)
            # belongs to head h; broadcasts per-head softmax denominators over
            # the feature rows of outT. Host-built (per-row memsets would need
            # unaligned start partitions, which compute engines disallow).
            S = consts.tile([HEADS, KT * P], F32R, name="S")
            nc.sync.dma_start(out=S, in_=sel_d[:, :])
            sums = consts.tile([HEADS, N], F32R, name="sums")
            recip = consts.tile([HEADS, N], F32R, name="recip")

            gamma_t = beta_t = None
            if apply_gamma:
                gamma_t = consts.tile([P, DIM], F32, name="gamma_t")
                g_ap = gamma_d[:]
                nc.sync.dma_start(
                    out=gamma_t,
                    in_=bass.AP(
                        tensor=g_ap.tensor, offset=g_ap.offset, ap=[[0, P]] + list(g_ap.ap)
                    ),
                )
            if apply_beta:
                beta_t = consts.tile([P, DIM], F32, name="beta_t")
                b_ap = beta_d[:]
                nc.sync.dma_start(
                    out=beta_t,
                    in_=bass.AP(
                        tensor=b_ap.tensor, offset=b_ap.offset, ap=[[0, P]] + list(b_ap.ap)
                    ),
                )

            xnT = bigp.tile([P, KT, N], F32R, name="xnT", tag="big")

            # ---- Phases A+B1 interleaved: LayerNorm+transpose for two token
            # tiles, then the V projection for those tiles — the V matmuls
            # keep the PE busy while the next tiles' LayerNorm runs on DVE/ACT.
            vts = []
            for jt in range(NT):
                vt = vpool.tile([P, HEADS * (DH + 1)], BF16, name=f"v{jt}", tag="v")
                vv = vt.rearrange("p (h c) -> p h c", c=DH + 1)
                nc.vector.memset(vv[:, :, DH : DH + 1], 1.0)
                vts.append((vt, vv))

            def emit_ln(it):
                xt = xpool.tile([P, DIM], F32, name=f"x{it}", tag="x")
                nc.sync.dma_start(out=xt, in_=x_d[it * P : (it + 1) * P, :])
                st = stats.tile([P, 2, 6], F32, name=f"st{it}", tag="st")
                nc.vector.bn_stats(out=st[:, 0], in_=xt[:, 0:512])
                nc.vector.bn_stats(out=st[:, 1], in_=xt[:, 512:1024])
                mv = stats.tile([P, 2], F32, name=f"mv{it}", tag="mv")
                nc.vector.bn_aggr(out=mv, in_=st)
                std = stats.tile([P, 1], F32, name=f"sd{it}", tag="sd")
                nc.scalar.activation(out=std, in_=mv[:, 1:2], func=AF.Sqrt, bias=eps_t)
                rstd = stats.tile([P, 1], F32, name=f"rs{it}", tag="rs")
                nc.vector.reciprocal(out=rstd, in_=std)
                nc.vector.tensor_scalar(
                    out=xt,
                    in0=xt,
                    scalar1=mv[:, 0:1],
                    scalar2=rstd,
                    op0=mybir.AluOpType.subtract,
                    op1=mybir.AluOpType.mult,
                )
                if gamma_t is not None:
                    nc.vector.tensor_mul(xt, xt, gamma_t)
                if beta_t is not None:
                    nc.vector.tensor_add(xt, xt, beta_t)
                for kt in range(KT):
                    pt = pspool.tile([P, P], F32, name=f"tp{it}_{kt}", tag="ps")
                    nc.tensor.transpose(pt, xt[:, kt * P : (kt + 1) * P], ident)
                    nc.vector.tensor_copy(xnT[:, kt, it * P : (it + 1) * P], pt)

            def emit_v_group(g, inserts=None):
                psv = [
                    pspool.tile([P, DIM], F32, name=f"psv{g}_{j}", tag="ps")
                    for j in range(2)
                ]
                for kt in range(KT):
                    wvt = wstream.tile([P, DIM], F32R, name=f"wv{g}_{kt}", tag="w")
                    nc.sync.dma_start(out=wvt, in_=wv_d[kt])
                    for j in range(2):
                        jt = 2 * g + j
                        for hf in range(2):
                            sl = slice(hf * 512, hf * 512 + 512)
                            nc.tensor.matmul(
                                psv[j][:, sl],
                                lhsT=xnT[:, kt, jt * P : (jt + 1) * P],
                                rhs=wvt[:, sl],
                                start=(kt == 0),
                                stop=(kt == KT - 1),
                            )
                    if inserts is not None:
                        for _ in range(4):
                            next(inserts, None)
                for j in range(2):
                    jt = 2 * g + j
                    vv = vts[jt][1]
                    for hf in range(2):
                        nc.vector.tensor_copy(
                            vv[:, hf * 8 : hf * 8 + 8, 0:DH],
                            psv[j][:, hf * 512 : hf * 512 + 512],
                        )

            # ---- Phases B2/C/D: QKV-qk, sim+bias, exp, AV -----------------
            # qT/kT for pair p+1 are computed INSIDE pair p's attention: two
            # projection matmuls are slotted into each (head, jt) step, filling
            # the PE bubbles left by the exp/mul chain so the PE stays dense
            # (keeps the HAM clock gate warm) and the projection phase costs
            # no extra wall-clock.
            wqs, qTs, kTs = [], [], []
            wq0 = wstream.tile([P, KT, 2 * P], F32R, name="wqk0", tag="w")
            nc.sync.dma_start(out=wq0, in_=wqk_d[0])
            wqs.append(wq0)
            qTs.append(qkpool.tile([P, N], F32R, name="qT0", tag="qk"))
            kTs.append(qkpool.tile([P, N], F32R, name="kT0", tag="qk"))

            def proj_steps(pn, which):
                """Generator: 16 matmuls (kt-outer, hf-inner) accumulating
                pair pn's q (which=0) or k (which=1) projection, then evicts
                to SBUF. Yields after each matmul."""
                ps = pspool.tile([P, N], F32, name=f"ps{'qk'[which]}{pn}", tag="ps")
                w0 = which * P
                for kt in range(KT):
                    for hf in range(2):
                        sl = slice(hf * 512, hf * 512 + 512)
                        nc.tensor.matmul(
                            ps[:, sl],
                            lhsT=wqs[pn][:, kt, w0 : w0 + P],
                            rhs=xnT[:, kt, sl],
                            start=(kt == 0),
                            stop=(kt == KT - 1),
                        )
                        yield
                nc.vector.tensor_copy((qTs, kTs)[which][pn], ps)
                while True:
                    yield

            for g in range(NT // 2):
                emit_ln(2 * g)
                emit_ln(2 * g + 1)
                emit_v_group(g)

            # Pair 0's projections run standalone (prologue).
            for which in range(2):
                g = proj_steps(0, which)
                for _ in range(17):  # 16 matmuls + the eviction copy
                    next(g)

            for p in range(PAIRS):
                qT, kTt = qTs[p], kTs[p]
                if p + 1 < PAIRS:
                    wq = wstream.tile(
                        [P, KT, 2 * P], F32R, name=f"wqk{p+1}", tag="w"
                    )
                    nc.sync.dma_start(out=wq, in_=wqk_d[p + 1])
                    wqs.append(wq)
                    qTs.append(
                        qkpool.tile([P, N], F32R, name=f"qT{p+1}", tag="qk")
                    )
                    kTs.append(
                        qkpool.tile([P, N], F32R, name=f"kT{p+1}", tag="qk")
                    )

                ot = opool.tile([P, N], F32R, name=f"outT{p}", tag="outT")
                if p == 0:
                    outTs = []
                outTs.append(ot)
                avs_t = [None, None]
                ets = [[], []]

                def emit_sim(hh, jt):
                    h = 2 * p + hh
                    hs = slice(hh * DH, (hh + 1) * DH)
                    bt = bpool.tile([P, N], BF16, name=f"b{h}_{jt}", tag="bias")
                    nc.sync.dma_start(out=bt, in_=bias_d[h, jt])
                    sim = pspool.tile([P, N], F32, name=f"sim{h}_{jt}", tag="ps")
                    for hf in range(2):
                        sl = slice(hf * 512, hf * 512 + 512)
                        nc.tensor.matmul(
                            sim[:, sl],
                            lhsT=kTt[hs, jt * P : (jt + 1) * P],
                            rhs=qT[hs, sl],
                            start=True,
                            stop=True,
                        )
                    et = epool.tile([P, N], BF16, name=f"e{h}_{jt}", tag="exp")
                    nc.scalar.activation(out=et, in_=sim, func=AF.Exp, scale=SCALE)
                    nc.vector.tensor_mul(et, et, bt)
                    ets[hh].append(et)

                def emit_av(hh, jt):
                    h = 2 * p + hh
                    for hf in range(2):
                        sl = slice(hf * 512, hf * 512 + 512)
                        nc.tensor.matmul(
                            avs_t[hh][:, sl],
                            lhsT=vts[jt][0][:, h * (DH + 1) : (h + 1) * (DH + 1)],
                            rhs=ets[hh][jt][:, sl],
                            start=(jt == 0),
                            stop=(jt == NT - 1),
                        )

                def emit_evict(hh):
                    h = 2 * p + hh
                    hs = slice(hh * DH, (hh + 1) * DH)
                    # Evict via SBUF staging (DMA cannot read PSUM; DVE cannot
                    # shift partitions — stage on matching partitions, then DMA
                    # to the head's row block in outT and its row in `sums`).
                    avs = xpool.tile([DH + 1, N], F32R, name=f"avs{h}", tag="avs")
                    nc.vector.tensor_copy(avs, avs_t[hh])
                    nc.sync.dma_start(out=outTs[p][hs, :], in_=avs[0:DH, :])
                    nc.sync.dma_start(out=sums[h : h + 1, :], in_=avs[DH : DH + 1, :])

                for hh in range(2):
                    avs_t[hh] = pspool.tile(
                        [DH + 1, N], F32, name=f"av{2*p+hh}", tag="ps"
                    )
                    # head 0 drives pair p+1's q projection, head 1 its k
                    # projection: 2 matmuls slotted into each jt step.
                    proj = (
                        proj_steps(p + 1, hh) if p + 1 < PAIRS else iter(())
                    )

                    def proj_step(k=2):
                        for _ in range(k):
                            next(proj, None)

                    emit_sim(hh, 0)
                    proj_step()
                    for jt in range(1, NT):
                        emit_sim(hh, jt)
                        emit_av(hh, jt - 1)
                        proj_step()
                    emit_av(hh, NT - 1)
                    proj_step(1)  # run the projection eviction
                    emit_evict(hh)

            # ---- Phase E: normalize outT by softmax denominators ----------
            with nc.allow_low_precision(
                reason="recip feeds an fp32r matmul; fp32r rounding intended"
            ):
                nc.vector.reciprocal(out=recip, in_=sums)
            for kt in range(KT):
                rs = pspool.tile([P, N], F32, name=f"rs{kt}", tag="ps")
                for hf in range(2):
                    sl = slice(hf * 512, hf * 512 + 512)
                    nc.tensor.matmul(
                        rs[:, sl],
                        lhsT=S[:, kt * P : (kt + 1) * P],
                        rhs=recip[:, sl],
                        start=True,
                        stop=True,
                    )
                nc.vector.tensor_mul(outTs[kt], outTs[kt], rs)

            # ---- Phase F: y = outT^T @ w_out ------------------------------
            wo_t = bigp.tile([P, KT, DIM], F32R, name="wo_t", tag="big")
            for kt in range(KT):
                nc.sync.dma_start(out=wo_t[:, kt, :], in_=wo_d[:, kt, :])
            for it in range(NT):
                yst = xpool.tile([P, DIM], F32, name=f"y{it}", tag="x")
                for hf in range(2):
                    sl = slice(hf * 512, hf * 512 + 512)
                    psy = pspool.tile([P, 512], F32, name=f"psy{it}_{hf}", tag="ps")
                    for kt in range(KT):
                        nc.tensor.matmul(
                            psy,
                            lhsT=outTs[kt][:, it * P : (it + 1) * P],
                            rhs=wo_t[:, kt, sl],
                            start=(kt == 0),
                            stop=(kt == KT - 1),
                        )
                    nc.vector.tensor_copy(yst[:, sl], psy)
                nc.sync.dma_start(out=y_d[it * P : (it + 1) * P, :], in_=yst)

    nc.compile()
    _BUILD_CACHE[key] = nc
    return nc


def _host_prep(ln_gamma, ln_beta, w_qkv, w_out, attn_bias):
    """Re-layout weights/bias for the device kernel (pure host-side reshapes)."""
    w_qkv = np.asarray(w_qkv, np.float32)
    w_out = np.asarray(w_out, np.float32)
    attn_bias = np.asarray(attn_bias, np.float32)

    wq_r = w_qkv[:, :INNER].reshape(KT, P, PAIRS, P).transpose(2, 1, 0, 3)
    wk_r = w_qkv[:, INNER : 2 * INNER].reshape(KT, P, PAIRS, P).transpose(2, 1, 0, 3)
    wqk = np.ascontiguousarray(np.concatenate([wq_r, wk_r], axis=3))
    wv = np.ascontiguousarray(w_qkv[:, 2 * INNER :].reshape(KT, P, DIM))
    wo = np.ascontiguousarray(w_out.reshape(KT, P, DIM).transpose(1, 0, 2))
    # exp(bias), transposed per head to [j, i]: the kernel multiplies it into
    # exp(sim) on the vector engine (exp(a+b) = exp(a)*exp(b)), keeping the
    # tensor engine free of bias-add matmuls.
    biasT = np.ascontiguousarray(
        np.exp(attn_bias[0].astype(np.float64)).astype(np.float32)
        .transpose(0, 2, 1)
        .reshape(HEADS, NT, P, N)
        .astype(ml_dtypes.bfloat16)
    )
    sel = np.zeros((HEADS, KT * P), dtype=np.float32)
    for h in range(HEADS):
        c0 = (h // 2) * P + (h % 2) * DH
        sel[h, c0 : c0 + DH] = 1.0
    in_map = {"wqk": wqk, "wv": wv, "wo": wo, "biasT": biasT, "sel": sel}

    gamma = np.asarray(ln_gamma, np.float32)
    beta = np.asarray(ln_beta, np.float32)
    apply_gamma = not np.all(gamma == 1.0)
    apply_beta = bool(np.any(beta != 0.0))
    if apply_gamma:
        in_map["gamma"] = gamma
    if apply_beta:
        in_map["beta"] = beta
    return in_map, apply_gamma, apply_beta


def kernel(x, ln_gamma, ln_beta, w_qkv, w_out, attn_bias):
    x = np.asarray(x, np.float32)
    in_map, apply_gamma, apply_beta = _host_prep(
        ln_gamma, ln_beta, w_qkv, w_out, attn_bias
    )
    nc = _build(apply_gamma, apply_beta)
    in_maps = [dict(in_map, x=np.ascontiguousarray(x[b])) for b in range(B)]
    res = run_bass_kernel_spmd(
        nc,
        in_maps,
        list(range(B)),
        trace=bool(int(os.environ.get("BA_TRACE", "0"))),
        tmpdir=os.environ.get("BA_TRACE_DIR") or None,
    )
    out = np.stack([res.results[i]["y"] for i in range(B)], axis=0)
    if bool(int(os.environ.get("BA_TRACE", "0"))):
        kernel.last_exec_time_ns = res.exec_time_ns
        kernel.last_mean_exec_time_ns = res.mean_exec_time_ns
    return out

